# revision 18
# baseline (speedup 1.0000x reference)
"""Trainium2 Bass kernel for nn_Block_1726576855578 (dense_mlp).

Sharding: 8-way data parallel over batch B=4096 (512 rows/core), all
weights replicated. Per-core pipeline (all layouts [feature-partition,
batch-free] so chunk ops stay partition-aligned):

  stage1: hT[mm, b] = W.T-tiled matmuls vs xT, bias via K=1 rank-update,
          evicted per-chunk into [81, 512] tiles with a ones row (row 80)
          so the chunk-linear bias rides in the K=81 matmul.
  stage2: per chunk c, t-tile j: y0/y1 = [120(t), 512(b)] psum matmuls,
          evicted to bf16, w = y0*y1 on DVE, rank-sum via 0/1 selector
          matmul accumulating z[80(s), 512(b)] in psum.
  merge:  signed-sqrt via ACT Abs/Sign/Sqrt + DVE mul; chunk L2 norm via
          ones-matmul on |z| (sum z_signed^2 == sum |z| exactly),
          1/max(sqrt, eps) on a [1,512] row, broadcast back with a K=1
          matmul, applied on DVE.
  stage3: out[b, o] psum accumulation over 20 chunk K-tiles (K=80) plus
          K=1 bias rank-update; evict + DMA.

Matmuls run as float32r (full PE rate at N>=256). The only bf16 in the
pipeline is the y0*y1 elementwise product path.
"""

import numpy as np

import concourse.bacc as bacc
import concourse.mybir as mybir
import concourse.tile as tile
from concourse import bass_utils
from concourse.bass import ts

F32 = mybir.dt.float32
F32R = mybir.dt.float32r
BF16 = mybir.dt.bfloat16
AF = mybir.ActivationFunctionType

NCORES = 8
B = 4096
BC = B // NCORES          # 512 rows per core
D = 2048                  # D0 == D1
MM = 1600
CHUNKS = 20
SIZE = 80                 # mm chunk width
RANK = 15
TDIM = SIZE * RANK        # 1200
TT = 120                  # t-tile width (10 tiles per chunk)
NTT = TDIM // TT          # 10
OUT = 3000
NO = 500                  # out free tile
NNT = OUT // NO           # 6
NBT = BC // 128           # 4 b-tiles
KD = D // 128             # 16 K-tiles over D
EPS = 1e-12

_NC = None


def _build_nc():
    nc = bacc.Bacc("TRN2", target_bir_lowering=False, debug=False,
                   num_devices=NCORES)

    x0t = nc.dram_tensor("x0t", [D, BC], F32R, kind="ExternalInput")
    x1t = nc.dram_tensor("x1t", [D, BC], F32R, kind="ExternalInput")
    w0t = nc.dram_tensor("w0t", [D, MM], F32R, kind="ExternalInput")
    w1t = nc.dram_tensor("w1t", [D, MM], F32R, kind="ExternalInput")
    mw0p = nc.dram_tensor("mw0p", [CHUNKS, SIZE + 1, TDIM], BF16,
                          kind="ExternalInput")
    mw1p = nc.dram_tensor("mw1p", [CHUNKS, SIZE + 1, TDIM], BF16,
                          kind="ExternalInput")
    wot = nc.dram_tensor("wot", [MM, OUT], BF16, kind="ExternalInput")
    boutr = nc.dram_tensor("boutr", [1, OUT], F32R, kind="ExternalInput")
    seld = nc.dram_tensor("seld", [2, TT, SIZE], BF16, kind="ExternalInput")
    onesd = nc.dram_tensor("onesd", [1, BC], F32R, kind="ExternalInput")
    onescol = nc.dram_tensor("onescol", [128, 1], F32R, kind="ExternalInput")
    onesb = nc.dram_tensor("onesb", [1, BC], BF16, kind="ExternalInput")
    outd = nc.dram_tensor("out", [BC, OUT], F32, kind="ExternalOutput")

    xdr = [x0t, x1t]
    wdr = [w0t, w1t]
    mwdr = [mw0p, mw1p]

    NMT = (MM + 127) // 128            # 13 mm tiles (last is 64 rows)
    mt_rows = [128] * 12 + [64]

    with tile.TileContext(nc) as tc:
        with (
            tc.tile_pool(name="const", bufs=1) as cpool,
            tc.tile_pool(name="hpool", bufs=1) as hpool,
        ):
            ones80 = cpool.tile([SIZE, 1], F32R, tag="ones80")
            nc.sync.dma_start(ones80[:], onescol[0:SIZE, :])
            ones1_80 = cpool.tile([1, SIZE], F32R, tag="ones1_80")
            nc.sync.dma_start(ones1_80[:], onesd[:, 0:SIZE])
            ones1_128 = cpool.tile([1, 128], F32R, tag="ones1_128")
            nc.sync.dma_start(ones1_128[:], onesd[:, 0:128])
            sel = [cpool.tile([TT, SIZE], BF16, tag=f"sel{p}", name=f"sel{p}")
                   for p in (0, 1)]
            nc.sync.dma_start(sel[0][:], seld[0])
            nc.sync.dma_start(sel[1][:], seld[1])
            bosb = cpool.tile([1, OUT], F32R, tag="bo")
            nc.sync.dma_start(bosb[:], boutr[:])

            # h chunk tiles, ones row prefilled once (row SIZE)
            h_tiles = [[None] * CHUNKS, [None] * CHUNKS]
            for side in (0, 1):
                for c in range(CHUNKS):
                    ht = hpool.tile([SIZE + 1, BC], BF16,
                                    tag=f"h{side}_{c}", name=f"h{side}_{c}")
                    nc.gpsimd.dma_start(ht[SIZE:SIZE + 1, :], onesb[:])
                    h_tiles[side][c] = ht

            zn_tiles = [None] * CHUNKS
            # chunk c's h rows live in packed tiles <= cend[c]
            cend = [(c * SIZE + SIZE - 1) // 128 for c in range(CHUNKS)]

            with (
                tc.tile_pool(name="xpool", bufs=1) as xpool,
                tc.tile_pool(name="wpool", bufs=3) as wpool,
                tc.tile_pool(name="hpk", bufs=1) as hpk,
                tc.tile_pool(name="mwpool", bufs=2) as mwpool,
                tc.tile_pool(name="wbpool", bufs=2) as wbpool,
                tc.tile_pool(name="spool", bufs=2) as spool,
                tc.tile_pool(name="rowpool", bufs=2) as rowpool,
                tc.tile_pool(name="ps1", bufs=1, space="PSUM") as ps1,
                tc.tile_pool(name="ypsum", bufs=2, space="PSUM") as ypsum,
                tc.tile_pool(name="zpsum", bufs=2, space="PSUM") as zpsum,
                tc.tile_pool(name="nrpsum", bufs=1, space="PSUM") as nrpsum,
            ):
                packed = [[None] * NMT, [None] * NMT]
                xs = [None, None]

                def emit_s1(side, mt):
                    rows = mt_rows[mt]
                    wcol = wpool.tile([128, KD * 128], F32R, tag="wcol",
                                      name=f"wcol{side}_{mt}")
                    nc.sync.dma_start(
                        wcol[:, 0:KD * rows],
                        wdr[side][:, mt * 128:mt * 128 + rows].rearrange(
                            "(k p) m -> p k m", p=128))
                    ps = ps1.tile([128, BC], F32, tag="s1", name=f"s1_{side}_{mt}")
                    for k in range(KD):
                        nc.tensor.matmul(
                            ps[0:rows, :], wcol[:, k * rows:(k + 1) * rows],
                            xs[side][:, ts(k, BC)],
                            start=(k == 0), stop=(k == KD - 1))
                    pk = hpk.tile([128, BC], BF16, tag=f"pk_{mt}",
                                  name=f"pk{side}_{mt}")
                    nc.any.tensor_copy(pk[0:rows, :], ps[0:rows, :])
                    packed[side][mt] = pk

                def emit_rechunk(side, c):
                    lo = c * SIZE
                    m0, r0 = lo // 128, lo % 128
                    n0 = min(128 - r0, SIZE)
                    nc.sync.dma_start(h_tiles[side][c][0:n0, :],
                                      packed[side][m0][r0:r0 + n0, :])
                    if n0 < SIZE:
                        nc.sync.dma_start(
                            h_tiles[side][c][n0:SIZE, :],
                            packed[side][m0 + 1][0:SIZE - n0, :])

                def emit_stage2(c):
                    mwsb = []
                    for side in (0, 1):
                        m = mwpool.tile([SIZE + 1, TDIM], BF16,
                                        tag=f"mw{side}", name=f"mw{side}_{c}")
                        nc.sync.dma_start(m[:], mwdr[side][c])
                        mwsb.append(m)
                    z_ps = zpsum.tile([SIZE, BC], F32, tag="z", name=f"z_{c}")
                    for j in range(NTT):
                        y0 = ypsum.tile([TT, BC], F32, tag="y0",
                                        name=f"y0_{c}_{j}")
                        nc.tensor.matmul(
                            y0[:], mwsb[0][:, ts(j, TT)],
                            h_tiles[0][c][:], start=True, stop=True)
                        y0b = wbpool.tile([TT, BC], BF16, tag="y0b",
                                          name=f"y0b_{c}_{j}")
                        nc.scalar.activation(y0b[:], y0[:], AF.Copy)
                        y1 = ypsum.tile([TT, BC], F32, tag="y1",
                                        name=f"y1_{c}_{j}")
                        nc.tensor.matmul(
                            y1[:], mwsb[1][:, ts(j, TT)],
                            h_tiles[1][c][:], start=True, stop=True)
                        wb = wbpool.tile([TT, BC], BF16, tag="wb",
                                         name=f"wb_{c}_{j}")
                        nc.vector.tensor_tensor(wb[:], y1[:], y0b[:],
                                                mybir.AluOpType.mult)
                        nc.tensor.matmul(z_ps[:], sel[j % 2][:], wb[:],
                                         start=(j == 0), stop=(j == NTT - 1))
                    # signed sqrt + chunk L2 norm
                    a = spool.tile([SIZE, BC], F32R, tag="a", name=f"a_{c}")
                    nc.scalar.activation(a[:], z_ps[:], AF.Abs)
                    sg = spool.tile([SIZE, BC], F32, tag="sg", name=f"sg_{c}")
                    nc.scalar.activation(sg[:], z_ps[:], AF.Sign)
                    sq = spool.tile([SIZE, BC], F32, tag="sq", name=f"sq_{c}")
                    nc.scalar.activation(sq[:], a[:], AF.Sqrt)
                    zs = spool.tile([SIZE, BC], F32, tag="zs", name=f"zs_{c}")
                    nc.gpsimd.tensor_tensor(zs[:], sg[:], sq[:],
                                            mybir.AluOpType.mult)
                    n2 = nrpsum.tile([1, BC], F32, tag="nr", name=f"n2_{c}")
                    nc.tensor.matmul(n2[:], ones80[:], a[:],
                                     start=True, stop=True)
                    sn = rowpool.tile([1, BC], F32, tag="sn", name=f"sn_{c}")
                    nc.scalar.activation(sn[:], n2[:], AF.Sqrt)
                    nc.vector.tensor_scalar_max(sn[:], sn[:], EPS)
                    rn = rowpool.tile([1, BC], F32, tag="rn", name=f"rn_{c}")
                    nc.vector.reciprocal_approx_fast(rn[:], sn[:])
                    rnr = rowpool.tile([1, BC], F32R, tag="rnr",
                                       name=f"rnr_{c}")
                    nc.vector.tensor_copy(rnr[:], rn[:])
                    rnb = nrpsum.tile([SIZE, BC], F32, tag="nr",
                                      name=f"rnb_{c}")
                    nc.tensor.matmul(rnb[:], ones1_80[:], rnr[:],
                                     start=True, stop=True)
                    zn = hpool.tile([SIZE, BC], BF16, tag=f"h0_{c}",
                                    name=f"zn_{c}")
                    nc.vector.tensor_mul(zn[:], zs[:], rnb[:])
                    zn_tiles[c] = zn

                # side 0 stage-1 fully; then side 1 interleaved with stage 2
                xs[0] = xpool.tile([128, KD * BC], F32R, tag="x", name="x0sb")
                nc.sync.dma_start(
                    xs[0][:], xdr[0].rearrange("(k p) b -> p k b", p=128))
                xs[1] = xpool.tile([128, KD * BC], F32R, tag="x1", name="x1sb")
                nc.sync.dma_start(
                    xs[1][:], xdr[1].rearrange("(k p) b -> p k b", p=128))
                for mt in range(NMT):
                    emit_s1(0, mt)
                    for c in range(CHUNKS):
                        if cend[c] == mt:
                            emit_rechunk(0, c)
                for mt in range(NMT):
                    emit_s1(1, mt)
                    for c in range(CHUNKS):
                        if cend[c] == mt:
                            emit_rechunk(1, c)
                            emit_stage2(c)

            # ---------------- stage 3: out = zn @ Wout.T + bout ----------
            with (
                tc.tile_pool(name="zpk", bufs=1) as zpk,
                tc.tile_pool(name="wopool", bufs=2) as wopool,
                tc.tile_pool(name="opool", bufs=2) as opool,
                tc.tile_pool(name="ps3", bufs=2, space="PSUM") as ps3,
            ):
                # pack zn chunks into [128, BC] K-tiles
                zp = []
                for mt in range(NMT):
                    rows = mt_rows[mt]
                    t = zpk.tile([128, BC], BF16, tag=f"zp_{mt}",
                                 name=f"zp_{mt}")
                    lo = mt * 128
                    while lo < mt * 128 + rows:
                        c, r = lo // SIZE, lo % SIZE
                        n = min(SIZE - r, mt * 128 + rows - lo)
                        nc.sync.dma_start(t[lo - mt * 128:lo - mt * 128 + n, :],
                                          zn_tiles[c][r:r + n, :])
                        lo += n
                    zp.append(t)
                for nt in range(NNT):
                    wo = wopool.tile([128, NMT * NO], BF16, tag="wo",
                                     name=f"wo_{nt}")
                    nc.sync.dma_start(
                        wo[:, 0:12 * NO],
                        wot[0:12 * 128, ts(nt, NO)].rearrange(
                            "(m p) o -> p m o", p=128))
                    nc.sync.dma_start(wo[0:64, 12 * NO:13 * NO],
                                      wot[12 * 128:MM, ts(nt, NO)])
                    for bt in range(NBT):
                        ops = ps3.tile([128, NO], F32, tag="o",
                                       name=f"o_{nt}_{bt}")
                        for mt in range(NMT):
                            rows = mt_rows[mt]
                            nc.tensor.matmul(
                                ops[:], zp[mt][0:rows, ts(bt, 128)],
                                wo[0:rows, ts(mt, NO)],
                                start=(mt == 0), stop=False)
                        nc.tensor.matmul(
                            ops[:], ones1_128[:],
                            bosb[0:1, ts(nt, NO)], start=False, stop=True)
                        osb = opool.tile([128, NO], F32, tag="ob",
                                         name=f"ob_{nt}_{bt}")
                        nc.any.tensor_copy(osb[:], ops[:])
                        nc.sync.dma_start(outd[ts(bt, 128), ts(nt, NO)],
                                          osb[:])

    nc.compile()
    return nc


def _get_nc():
    global _NC
    if _NC is None:
        _NC = _build_nc()
    return _NC


def _prep_inputs(x0, x1, W0, b0, W1, b1, mW0, mb0, mW1, mb1, Wout, bout):
    import ml_dtypes
    f = np.float32
    # fold the stage-1 bias through the chunk linear:
    # y = mW @ (h_raw + b0_chunk) + mb = mW @ h_raw + (mW @ b0_chunk + mb)
    mb0f = np.einsum("cts,cs->ct", mW0, b0.reshape(CHUNKS, SIZE)) + mb0
    mb1f = np.einsum("cts,cs->ct", mW1, b1.reshape(CHUNKS, SIZE)) + mb1
    shared = {
        "w0t": np.ascontiguousarray(W0.T, dtype=f),
        "w1t": np.ascontiguousarray(W1.T, dtype=f),
        "mw0p": np.concatenate(
            [mW0.transpose(0, 2, 1), mb0f[:, None, :]],
            axis=1).astype(ml_dtypes.bfloat16),
        "mw1p": np.concatenate(
            [mW1.transpose(0, 2, 1), mb1f[:, None, :]],
            axis=1).astype(ml_dtypes.bfloat16),
        "wot": np.ascontiguousarray(Wout.T).astype(ml_dtypes.bfloat16),
        "boutr": np.ascontiguousarray(bout.reshape(1, OUT), dtype=f),
    }
    k = np.arange(TT)
    s = np.arange(SIZE)
    selm = np.stack([
        ((40 * p + k[:, None]) % SIZE == s[None, :]) for p in (0, 1)
    ]).astype(ml_dtypes.bfloat16)
    shared["seld"] = selm
    shared["onesd"] = np.ones((1, BC), dtype=f)
    shared["onesb"] = np.ones((1, BC), dtype=ml_dtypes.bfloat16)
    shared["onescol"] = np.ones((128, 1), dtype=f)
    x0t = np.ascontiguousarray(x0.T, dtype=f)
    x1t = np.ascontiguousarray(x1.T, dtype=f)
    in_maps = []
    for c in range(NCORES):
        m = dict(shared)
        m["x0t"] = np.ascontiguousarray(x0t[:, c * BC:(c + 1) * BC])
        m["x1t"] = np.ascontiguousarray(x1t[:, c * BC:(c + 1) * BC])
        in_maps.append(m)
    return in_maps


_RUNNER = None


def _get_runner():
    """Build the sharded PJRT executable once and reuse it across calls."""
    global _RUNNER
    if _RUNNER is not None:
        return _RUNNER
    import jax
    from jax.sharding import Mesh, NamedSharding, PartitionSpec
    from jax.experimental.shard_map import shard_map
    from concourse.bass2jax import (
        _bass_exec_p, install_neuronx_cc_hook, partition_id_tensor)

    nc = _get_nc()
    install_neuronx_cc_hook()

    in_names, out_names, out_avals, zero_outs = [], [], [], []
    pname = nc.partition_id_tensor.name if nc.partition_id_tensor else None
    for alloc in nc.m.functions[0].allocations:
        if not isinstance(alloc, mybir.MemoryLocationSet):
            continue
        name = alloc.memorylocations[0].name
        if alloc.kind == "ExternalInput":
            if name != pname:
                in_names.append(name)
        elif alloc.kind == "ExternalOutput":
            shape = tuple(alloc.tensor_shape)
            dtype = mybir.dt.np(alloc.dtype)
            out_names.append(name)
            out_avals.append(jax.core.ShapedArray(shape, dtype))
            zero_outs.append(np.zeros(shape, dtype))
    n_params = len(in_names)
    all_names = in_names + out_names
    if pname is not None:
        all_names.append(pname)

    def _body(*args):
        operands = list(args)
        if pname is not None:
            operands.append(partition_id_tensor())
        return tuple(_bass_exec_p.bind(
            *operands,
            out_avals=tuple(out_avals),
            in_names=tuple(all_names),
            out_names=tuple(out_names),
            lowering_input_output_aliases=(),
            sim_require_finite=True,
            sim_require_nnan=True,
            nc=nc,
        ))

    devices = jax.devices()[:NCORES]
    mesh = Mesh(np.asarray(devices), ("core",))
    nin = n_params + len(out_names)
    fn = jax.jit(
        shard_map(_body, mesh=mesh,
                  in_specs=(PartitionSpec("core"),) * nin,
                  out_specs=(PartitionSpec("core"),) * len(out_names),
                  check_rep=False),
        keep_unused=True)
    sharding = NamedSharding(mesh, PartitionSpec("core"))
    zeros_dev = [jax.device_put(
        np.concatenate([z] * NCORES, axis=0), sharding) for z in zero_outs]
    _RUNNER = (fn, in_names, out_names, zeros_dev, sharding)
    return _RUNNER


def _put_inputs(in_maps):
    import jax
    fn, in_names, out_names, zeros_dev, sharding = _get_runner()
    return [jax.device_put(
        np.concatenate([in_maps[c][n] for c in range(NCORES)], axis=0),
        sharding) for n in in_names]


def _run(dev_in):
    import jax
    fn, in_names, out_names, zeros_dev, sharding = _get_runner()
    outs = fn(*dev_in, *zeros_dev)
    jax.block_until_ready(outs)
    return outs


def kernel(**inputs):
    inputs = {k: np.asarray(v, dtype=np.float32) for k, v in inputs.items()}
    in_maps = _prep_inputs(**inputs)
    dev_in = _put_inputs(in_maps)
    outs = _run(dev_in)
    full = np.asarray(outs[0])          # [NCORES*BC, OUT] concat over cores
    return np.ascontiguousarray(full.reshape(B, OUT))
